# revision 29
# baseline (speedup 1.0000x reference)
"""Trainium2 Bass kernel for nn_CAModule (channel attention, sparse_attention).

Reference computation per batch b (x: [16, 512, 64, 64] f32, beta: [1] f32):
    q = x[b].reshape(512, 4096)              # [C, N]
    energy = q @ q.T                         # [C, C]   (symmetric!)
    att = softmax(max_j(energy) - energy)    # row-wise, == softmax(-energy)
    out[b] = beta * (att @ q)                # [C, N]

Sharding: data-parallel over batch, 2 batches per core on 8 cores.

Key tricks:
  - softmax(max - energy) == exp(mn_i - e_ij)/Z_i with mn_i = row min
    (shift invariance; mn is the max of the softmax argument).
  - energy is symmetric, so att^T (needed as the stationary operand of the
    second matmul) is computed directly from the energy tiles: the tile of
    rows jc is also the tile of columns jc. Only q itself needs a physical
    512x4096 transpose (done on the PE via identity matmuls).
  - mn_i is subtracted along the *free* dim of the transposed tiles by a
    K=1 accumulating matmul ((-1s) x mnT) into the energy PSUM banks.
  - matmuls run as float32r (e8m11, RNE-on-write, exact PE): 1 cycle/row
    vs 4 for f32. Mode "split" decomposes q = h + l (h = f32r(q)) and runs
    E = h@hT + h@lT + l@hT for ~fp32 accuracy at 3 passes.
  - q is loaded in 512-column pieces, channel-chunk round-robin, so the
    transpose/energy pipeline starts after ~1 MB instead of 8 MB.

Modes via CAM_MODE env: "f32r" (default), "split", "f32".
"""
import os
import sys

sys.path.insert(0, "/opt/trn_rl_repo")

import numpy as np  # noqa: E402

try:
    import jax

    jax.config.update("jax_compilation_cache_dir", "/tmp/jax_cc_cache")
    jax.config.update("jax_persistent_cache_min_compile_time_secs", 0.0)
except Exception:
    pass

import concourse.bass as bass  # noqa: E402
import concourse.bacc as bacc  # noqa: E402
import concourse.mybir as mybir  # noqa: E402
from concourse.tile import TileContext  # noqa: E402
from concourse.masks import make_identity  # noqa: E402
from concourse.bass_utils import run_bass_kernel_spmd  # noqa: E402

F32 = mybir.dt.float32
F32R = mybir.dt.float32r
AX = mybir.AxisListType
OP = mybir.AluOpType
AF = mybir.ActivationFunctionType

B, C, HH, WW = 16, 512, 64, 64
N = HH * WW          # 4096
P = 128
NCORES = 8
BPC = B // NCORES    # 2 batches per core
CC = C // P          # 4 channel chunks
NT = N // P          # 32 spatial chunks (transpose granularity)
NF = N // 512        # 8 q pieces / output free-dim chunks
TPP = 512 // P       # t-chunks per q piece (4)

MODE = os.environ.get("CAM_MODE", "split")

# energy upper-triangle: per ic, compute columns j >= JSTART[ic], mirror rest
JSTART = [0, 128, 256, 256]
MIRROR_PAIRS = [(0, 1), (0, 2), (0, 3), (1, 2), (1, 3)]


def build_nc(mode: str, bpc: int = BPC, reps: int = 1):
    nc = bacc.Bacc(None, target_bir_lowering=False)
    xs = nc.dram_tensor("xs", [bpc, C, N], F32, kind="ExternalInput")
    beta = nc.dram_tensor("beta", [1, 1], F32, kind="ExternalInput")
    ys = nc.dram_tensor("ys", [bpc, C, N], F32, kind="ExternalOutput")

    # matmul dtype for the two big matmuls
    MMDT = F32 if mode == "f32" else F32R
    # dtype in which q is loaded / transposed. NOTE: in "split" this must
    # stay F32 — the hardware f32r transpose path rounds the moving data to
    # 11 mantissa bits (verified empirically: absmax err jumps 7.8e-3), which
    # destroys the h/l error-compensation.
    QDT = F32R if mode == "f32r" else F32
    # dtype of the mn-fold matmul operands (exact f32 unless pure-f32r mode)
    NDT = F32R if mode == "f32r" else F32

    with TileContext(nc) as tc:
        with (
            tc.tile_pool(name="consts", bufs=1) as consts,
            tc.tile_pool(name="pq", bufs=(36 if mode == "split" else 64)) as pq,
            tc.tile_pool(name="pqr", bufs=32) as pqr,
            tc.tile_pool(name="pqt", bufs=8) as pqt,
            tc.tile_pool(name="pexpt", bufs=8) as pexpt,
            tc.tile_pool(name="pscr", bufs=2) as pscr,
            tc.tile_pool(name="posb", bufs=4) as posb,
            tc.tile_pool(name="pstat", bufs=2) as pstat,
            tc.tile_pool(name="pse", bufs=4, space="PSUM") as pse,
            tc.tile_pool(name="psg", bufs=2, space="PSUM") as psg,
            tc.tile_pool(name="pso", bufs=2, space="PSUM") as pso,
        ):
            # ---- constants ----
            ident = consts.tile([P, P], F32)
            make_identity(nc, ident)
            if QDT == F32R:
                identq = consts.tile([P, P], F32R)
                nc.vector.tensor_copy(identq, ident)
            else:
                identq = ident
            ones1 = consts.tile([1, P], F32)
            nc.vector.memset(ones1, 1.0)
            negones_f = consts.tile([1, P], F32)
            nc.vector.memset(negones_f, -1.0)
            if NDT == F32R:
                negones = consts.tile([1, P], F32R)
                nc.vector.tensor_copy(negones, negones_f)
            else:
                negones = negones_f

            # beta broadcast to [P, 1]
            beta_sb = consts.tile([1, 1], F32)
            nc.sync.dma_start(beta_sb, beta[:, :])
            ps_b = psg.tile([P, 1], F32, tag="g")
            nc.tensor.matmul(ps_b, ones1, beta_sb, start=True, stop=True)
            beta_bc = consts.tile([P, 1], F32)
            nc.vector.tensor_copy(beta_bc, ps_b)

            for b_rep in range(bpc * reps):
                b = b_rep % bpc
                # ---- load q in pieces, channel-chunk round-robin ----
                Q = [[None] * NF for _ in range(CC)]
                for p in range(NF):
                    for c in range(CC):
                        q = pq.tile([P, 512], QDT, tag="q", name=f"q{b_rep}_{c}_{p}")
                        src = xs[b, P * c : P * (c + 1), 512 * p : 512 * (p + 1)]
                        if mode == "f32r":
                            nc.gpsimd.dma_start(q, src)  # SWDGE cast f32->f32r
                        else:
                            nc.sync.dma_start(q, src)
                        Q[c][p] = q
                if mode == "split":
                    # filled inline at each piece's last transpose (keeps the
                    # in-order DVE stream from blocking on future loads)
                    Qr = [[None] * NF for _ in range(CC)]
                else:
                    Qr = Q

                # ---- energy: E[ic] = (q @ q.T)[ic-chunk, :] via transposed tiles ----
                E = [
                    pse.tile([P, 512], F32, tag="e", name=f"E{b_rep}_{i}")
                    for i in range(CC)
                ]
                # software-pipelined by one stage: transposes + DVE split of
                # t+1 are emitted before the matmuls of t, so the cross-engine
                # (PE -> DVE -> PE) latency hides under the matmuls.
                def emit_stage1(t):
                    p, o = t // TPP, (t % TPP) * P
                    stg = psg.tile([P, 512], QDT, tag="g", name=f"stg{b_rep}_{t}")
                    for c in range(CC):
                        nc.tensor.transpose(
                            stg[:, P * c : P * (c + 1)],
                            Q[c][p][:, o : o + P],
                            identq,
                        )
                    if mode == "split" and t % TPP == TPP - 1:
                        for c in range(CC):
                            qr = pqr.tile(
                                [P, 512], F32R, tag="qr", name=f"qr{b_rep}_{c}_{p}"
                            )
                            # ACT is idle here; keep DVE for the ht/lt chain
                            nc.scalar.copy(qr, Q[c][p])
                            Qr[c][p] = qr
                    if mode == "split":
                        ht = pqt.tile([P, 512], F32R, tag="ht", name=f"ht{b_rep}_{t}")
                        lt = pqt.tile([P, 512], F32R, tag="lt", name=f"lt{b_rep}_{t}")
                        nc.vector.tensor_copy(ht, stg)          # h = rne11(q)
                        nc.vector.tensor_tensor(
                            lt, stg, ht.bitcast(F32), op=OP.subtract
                        )                                        # l = q - h
                        return [(ht, ht), (ht, lt), (lt, ht)]
                    qt = pqt.tile([P, 512], MMDT, tag="qt", name=f"qt{b_rep}_{t}")
                    nc.vector.tensor_copy(qt, stg)
                    return [(qt, qt)]

                # upper-triangle only: E[ic] columns j >= JSTART[ic]
                # (ic=3 widened to 256 cols: f32r needs free >= 256 for
                # 1 cyc/row, so block (3,2) is computed directly instead
                # of mirrored)
                def emit_stage2(t, ops):
                    for oi, (L, R) in enumerate(ops):
                        for ic in range(CC):
                            js = JSTART[ic]
                            nc.tensor.matmul(
                                E[ic][:, js:],
                                L[:, P * ic : P * (ic + 1)],
                                R[:, js:],
                                start=(t == 0 and oi == 0),
                                stop=(t == NT - 1 and oi == len(ops) - 1),
                            )

                prev = (0, emit_stage1(0))
                for t in range(1, NT):
                    ops = emit_stage1(t)
                    emit_stage2(*prev)
                    prev = (t, ops)
                emit_stage2(*prev)

                # ---- mirror lower-triangle blocks: E[j][:, i] = E[i][:, j]^T ----
                for (ui, uj) in MIRROR_PAIRS:
                    blk = pstat.tile(
                        [P, P], F32, tag="mir", name=f"mir{b_rep}_{ui}_{uj}"
                    )
                    nc.vector.tensor_copy(blk, E[ui][:, P * uj : P * (uj + 1)])
                    nc.tensor.matmul(
                        E[uj][:, P * ui : P * (ui + 1)],
                        blk,
                        ident,
                        is_transpose=True,
                        start=False,
                        stop=True,
                        skip_group_check=True,
                    )

                # ---- row stats: mn = rowmin(E), Z = sum_j exp(mn - e) ----
                mn = pstat.tile([P, CC], F32, tag="mn")
                for ic in range(CC):
                    nc.vector.tensor_reduce(
                        mn[:, ic : ic + 1], E[ic], axis=AX.X, op=OP.min
                    )
                if mode == "f32r":
                    mnv = pstat.tile([P, CC], F32R, tag="mnv")
                    nc.vector.tensor_copy(mnv, mn)  # rne11 so matmul sees same value
                    mn_bias = mnv.bitcast(F32)
                    tsrc = mnv
                else:
                    mn_bias = mn
                    tsrc = mn

                Z = pstat.tile([P, CC], F32, tag="z")
                for ic in range(CC):
                    scr = pscr.tile([P, 512], F32, tag="scr")
                    nc.scalar.activation(
                        scr,
                        E[ic],
                        AF.Exp,
                        bias=mn_bias[:, ic : ic + 1],
                        scale=-1.0,
                        accum_out=Z[:, ic : ic + 1],
                    )

                # ---- mnT: [1, 512] row vector of mn ----
                ps_s = psg.tile([CC, P], NDT, tag="g", padded_shape=[P, 512])
                identm = identq if mode == "f32r" else ident
                nc.tensor.matmul(
                    ps_s, tsrc, identm, is_transpose=True, start=True, stop=True
                )
                sbs = pstat.tile([CC, P], NDT, tag="sbs")
                nc.vector.tensor_copy(sbs, ps_s)
                mnT = pstat.tile([1, C], NDT, tag="mnT")
                for c in range(CC):
                    nc.sync.dma_start(
                        mnT[0:1, P * c : P * (c + 1)], sbs[c : c + 1, :]
                    )

                # ---- fold -mn along free dim into E (E' = e[j,i] - mn_i) ----
                for ic in range(CC):
                    nc.tensor.matmul(
                        E[ic], negones, mnT,
                        start=False, stop=True, skip_group_check=True,
                    )

                # ---- att^T tiles: expT[jc][j, i] = exp(mn_i - e[j, i]) ----
                expT = []
                for jc in range(CC):
                    eT = pexpt.tile([P, C], MMDT, tag="expt", name=f"eT{b_rep}_{jc}")
                    nc.scalar.activation(eT, E[jc], AF.Exp, scale=-1.0)
                    expT.append(eT)

                # ---- scale vector: rZb = beta / Z ----
                rZ = pstat.tile([P, CC], F32, tag="rz")
                nc.vector.reciprocal(rZ, Z)
                rZb = pstat.tile([P, CC], F32, tag="rzb")
                nc.vector.tensor_tensor(
                    rZb, rZ, beta_bc.broadcast_to([P, CC]), op=OP.mult
                )

                # ---- out = rZb * (expT.T @ q) ----
                for ic in range(CC):
                    for nf in range(NF):
                        po = pso.tile([P, 512], F32, tag="o")
                        for jc in range(CC):
                            nc.tensor.matmul(
                                po,
                                expT[jc][:, P * ic : P * (ic + 1)],
                                Qr[jc][nf],
                                start=(jc == 0),
                                stop=(jc == CC - 1),
                            )
                        ob = posb.tile([P, 512], F32, tag="osb")
                        nc.scalar.activation(
                            ob, po, AF.Copy, scale=rZb[:, ic : ic + 1]
                        )
                        nc.sync.dma_start(
                            ys[b, P * ic : P * (ic + 1), 512 * nf : 512 * (nf + 1)],
                            ob,
                        )
    nc.finalize()
    return nc


_NC_CACHE = {}


def _get_nc(mode: str, bpc: int = BPC, reps: int = 1):
    key = (mode, bpc, reps)
    if key not in _NC_CACHE:
        _NC_CACHE[key] = build_nc(mode, bpc, reps)
    return _NC_CACHE[key]


def kernel(x: np.ndarray, beta: np.ndarray) -> np.ndarray:
    x = np.ascontiguousarray(np.asarray(x, dtype=np.float32))
    beta2 = np.asarray(beta, dtype=np.float32).reshape(1, 1)
    assert x.shape == (B, C, HH, WW)
    xf = x.reshape(B, C, N)

    nc = _get_nc(MODE)
    in_maps = [
        {"xs": xf[k * BPC : (k + 1) * BPC], "beta": beta2} for k in range(NCORES)
    ]
    res = run_bass_kernel_spmd(nc, in_maps, list(range(NCORES)))
    out = np.concatenate([r["ys"] for r in res.results], axis=0)
    return out.reshape(B, C, HH, WW).astype(np.float32, copy=False)


if __name__ == "__main__":
    rng = np.random.default_rng(0)
    x = rng.standard_normal((B, C, HH, WW), dtype=np.float32)
    beta = rng.standard_normal(1).astype(np.float32)
    y = kernel(x=x, beta=beta)
    print("out", y.shape, y.dtype, float(np.abs(y).max()))
